# revision 1
# baseline (speedup 1.0000x reference)
"""Multi-head attention (B=2, S=2048, D=1024, H=16) on 8 TRN2 NeuronCores.

Sharding: (batch, head-group) SPMD. Core c handles batch b = c//4 and local
heads [4*(c%4), 4*(c%4)+4). Each core computes its 4 heads' attention plus the
partial o-projection (row-parallel over the head dimension); the host sums the
4 partial outputs per batch (the all-reduce of row-parallel o_proj) and adds
b_o.

Device dataflow per core (all matmuls fp32r except P@V in bf16):
  phase 1: QT = (wq/8) @ x.T, KT = wk @ x.T   (transposed layouts [ch, seq])
           V  = x @ wv.T                       ([seq, ch] chunks, bf16, with a
                                                ones column per head for the
                                                softmax denominator)
  phase 2: per q-block (512) and k-tile (128):
           T = S.T block   = KT_h.T @ QT_h     (PSUM, K=64 row-packed pairs)
           E = exp(T)                          (ACT, PSUM->SBUF bf16)
           P.T = E * maskT                     (DVE, bf16 2x mode)
           Cq += [V_h|1].T @ P.T               (PSUM accumulate, row 64 = den)
           then rec = exp(-ln(den)), partition-broadcast, Cn = Cq * rec,
           y.T += woT_h.T @ Cn                 (fp32r), DMA out.
"""
import os
import sys

if "/opt/trn_rl_repo" not in sys.path:
    sys.path.insert(0, "/opt/trn_rl_repo")
os.environ.setdefault("JAX_PLATFORMS", "axon,cpu")

from contextlib import ExitStack

import ml_dtypes
import numpy as np

import concourse.bass as bass
import concourse.tile as tile
from concourse import bacc, library_config, mybir
from concourse.bass_utils import run_bass_kernel_spmd

F32 = mybir.dt.float32
F32R = mybir.dt.float32r
BF16 = mybir.dt.bfloat16
EXP = mybir.ActivationFunctionType.Exp
LN = mybir.ActivationFunctionType.Ln

B, S, D = 2, 2048, 1024
H, HD = 16, 64
HL = 4            # local heads per core
CH = HL * HD      # 256 local channels
N_CORES = 8
KC = D // 128     # 8 contraction chunks for the projections
NQB = S // 512    # 4 q blocks
NKT = S // 128    # 16 k tiles

_CACHE = {}


def _build_nc(debug=False):
    nc = bacc.Bacc("TRN2", target_bir_lowering=False)
    xT_d = nc.declare_dram_parameter("xT", [D, S], F32R, isOutput=False)
    mk_d = nc.declare_dram_parameter("maskT", [S, S], BF16, isOutput=False)
    wqT_d = nc.declare_dram_parameter("wqT", [D, CH], F32R, isOutput=False)
    wkT_d = nc.declare_dram_parameter("wkT", [D, CH], F32R, isOutput=False)
    wvT_d = nc.declare_dram_parameter("wvT", [D, CH], F32R, isOutput=False)
    woT_d = nc.declare_dram_parameter("woT", [CH, D], BF16, isOutput=False)
    yT_d = nc.declare_dram_parameter("yT", [D, S], F32, isOutput=True)
    dbg = {}
    if debug:
        dbg["qt0"] = nc.declare_dram_parameter("d_qt0", [128, S], F32, isOutput=True)
        dbg["kt0"] = nc.declare_dram_parameter("d_kt0", [128, S], F32, isOutput=True)
        dbg["v0"] = nc.declare_dram_parameter("d_v0", [128, HL * 65], BF16, isOutput=True)
        dbg["tq"] = nc.declare_dram_parameter("d_tq", [128, 1024], F32, isOutput=True)
        dbg["ex"] = nc.declare_dram_parameter("d_ex", [128, 1024], BF16, isOutput=True)
        dbg["pt"] = nc.declare_dram_parameter("d_pt", [128, 1024], BF16, isOutput=True)
        dbg["cq"] = nc.declare_dram_parameter("d_cq", [65, HL * 512], F32, isOutput=True)
        dbg["rec"] = nc.declare_dram_parameter("d_rec", [1, HL * 512], F32, isOutput=True)
        dbg["recb"] = nc.declare_dram_parameter("d_recb", [64, HL * 512], F32, isOutput=True)
        dbg["cn"] = nc.declare_dram_parameter("d_cn", [64, HL * 512], F32, isOutput=True)

    with tile.TileContext(nc) as tc, ExitStack() as ctx:
        nc.gpsimd.load_library(library_config.attn)
        const = ctx.enter_context(tc.tile_pool(name="const", bufs=1))
        psum = ctx.enter_context(tc.tile_pool(name="psum", bufs=1, space="PSUM"))

        # ---- resident tensors ----
        mk = [const.tile([128, S], BF16, name=f"mk{kt}") for kt in range(NKT)]
        wo = []
        for h in range(HL):
            t = const.tile([64, D], BF16, name=f"wo{h}")
            nc.sync.dma_start(t[:], woT_d[h * 64:(h + 1) * 64, :])
            wo.append(t)
        # persistent QT/KT ([2 heads * 64 d, seq] pair tiles) and V chunks
        qt = [const.tile([128, S], BF16, name=f"qt{i}") for i in range(2)]
        kt_sb = [const.tile([128, S], BF16, name=f"kt{i}") for i in range(2)]
        v_sb = [const.tile([128, HL * 65], BF16, name=f"v{i}") for i in range(NKT)]
        for st in range(NKT):
            # ones column per head (softmax denominator trick)
            nc.gpsimd.memset(
                v_sb[st].rearrange("p (h c) -> p h c", h=HL)[:, :, 64:65], 1.0
            )

        # ---- phase 1: projections (own pool, closed before phase 2) ----
        with tc.tile_pool(name="p1", bufs=1) as p1:
            wq_sb = [p1.tile([128, CH], F32R, name=f"wq{k}") for k in range(KC)]
            wk_sb = [p1.tile([128, CH], F32R, name=f"wk{k}") for k in range(KC)]
            wv_sb = [p1.tile([128, CH], F32R, name=f"wv{k}") for k in range(KC)]
            for k in range(KC):
                nc.sync.dma_start(wq_sb[k][:], wqT_d[k * 128:(k + 1) * 128, :])
                nc.sync.dma_start(wk_sb[k][:], wkT_d[k * 128:(k + 1) * 128, :])
                nc.sync.dma_start(wv_sb[k][:], wvT_d[k * 128:(k + 1) * 128, :])

            for qh in range(4):  # seq quarters of 512
                xt = []
                for k in range(KC):
                    t = p1.tile([128, 512], F32R, name=f"xt{k}", bufs=1)
                    nc.sync.dma_start(
                        t[:], xT_d[k * 128:(k + 1) * 128, qh * 512:(qh + 1) * 512]
                    )
                    xt.append(t)

                # interleave Q/K m-tiles with V seq-tiles for PE overlap
                for j, (wsb, dst, mt) in enumerate(
                    [(wq_sb, qt, 0), (wq_sb, qt, 1), (wk_sb, kt_sb, 0), (wk_sb, kt_sb, 1)]
                ):
                    ps = psum.tile([128, 512], F32, name="psa", tag="psa", bufs=2)
                    for k in range(KC):
                        nc.tensor.matmul(
                            ps[:],
                            wsb[k][:, mt * 128:(mt + 1) * 128],
                            xt[k][:],
                            start=(k == 0), stop=(k == KC - 1),
                        )
                    nc.scalar.copy(dst[mt][:, qh * 512:(qh + 1) * 512], ps[:])
                    if j % 2 == 0:  # 2 V seq-tiles after every other QK job
                        for st_l in range(2):
                            sl = j + st_l
                            st = qh * 4 + sl
                            vp = psum.tile([128, CH], F32, name="psb", tag="psb", bufs=1)
                            for k in range(KC):
                                nc.tensor.matmul(
                                    vp[:],
                                    xt[k][:, sl * 128:(sl + 1) * 128],
                                    wv_sb[k][:],
                                    start=(k == 0), stop=(k == KC - 1),
                                )
                            nc.vector.tensor_copy(
                                v_sb[st].rearrange("p (h c) -> p h c", h=HL)[:, :, 0:64],
                                vp.rearrange("p (h c) -> p h c", h=HL),
                            )

        # mask loads issued after phase-1 inputs: first consumer is phase 2
        for kt in range(NKT):
            nc.sync.dma_start(mk[kt][:], mk_d[kt * 128:(kt + 1) * 128, :])

        if debug:
            nc.gpsimd.dma_start(dbg["qt0"][:], qt[0][:])
            nc.gpsimd.dma_start(dbg["kt0"][:], kt_sb[0][:])
            nc.sync.dma_start(dbg["v0"][:], v_sb[0][:])

        # ---- phase 2: attention + o_proj ----
        with tc.tile_pool(name="work", bufs=1) as work:
            for qb in range(NQB):
                cq = psum.tile([128, HL * 512], F32, name="psb", tag="psb", bufs=1)
                for ktile in range(NKT):
                    for pair in range(2):
                        tq = psum.tile([128, 1024], F32, name="psa", tag="psa", bufs=2)
                        for hh in range(2):
                            nc.tensor.matmul(
                                tq[:, hh * 512:(hh + 1) * 512],
                                kt_sb[pair][hh * 64:(hh + 1) * 64,
                                            ktile * 128:(ktile + 1) * 128],
                                qt[pair][hh * 64:(hh + 1) * 64,
                                         qb * 512:(qb + 1) * 512],
                                start=True, stop=True,
                            )
                        if debug and qb == 0 and ktile == 0 and pair == 0:
                            tqc = work.tile([128, 1024], F32, name="tqc", tag="ysb2", bufs=1)
                            nc.vector.tensor_copy(tqc[:], tq[:])
                            nc.sync.dma_start(dbg["tq"][:], tqc[:])
                        ex = work.tile([128, 1024], BF16, name="expq", tag="expq", bufs=4)
                        nc.scalar.activation(ex[:], tq[:], EXP)
                        if debug and qb == 0 and ktile == 0 and pair == 0:
                            nc.sync.dma_start(dbg["ex"][:], ex[:])
                        pt = work.tile([128, 1024], BF16, name="pt", tag="pt", bufs=6)
                        for hh in range(2):
                            nc.vector.tensor_mul(
                                pt[:, hh * 512:(hh + 1) * 512],
                                ex[:, hh * 512:(hh + 1) * 512],
                                mk[ktile][:, qb * 512:(qb + 1) * 512],
                            )
                        if debug and qb == 0 and ktile == 0 and pair == 0:
                            nc.sync.dma_start(dbg["pt"][:], pt[:])
                        for hh in range(2):
                            h = pair * 2 + hh
                            nc.tensor.matmul(
                                cq[0:65, h * 512:(h + 1) * 512],
                                v_sb[ktile][:, h * 65:h * 65 + 65],
                                pt[:, hh * 512:(hh + 1) * 512],
                                start=(ktile == 0), stop=(ktile == NKT - 1),
                            )
                # softmax denominator: rec = exp(-ln(den)) = 1/den
                if debug and qb == 0:
                    cqc = work.tile([65, HL * 512], F32, name="cqc", tag="ysb2", bufs=1)
                    nc.vector.tensor_copy(cqc[:], cq[0:65, :])
                    nc.sync.dma_start(dbg["cq"][:], cqc[:])
                nc.scalar.activation(cq[64:65, :], cq[64:65, :], LN)
                rec = work.tile([65, HL * 512], F32, name="rec", tag="cn", bufs=2)
                nc.scalar.activation(rec[64:65, :], cq[64:65, :], EXP, scale=-1.0)
                # hop the row to partition 0 via DMA: partition_broadcast's
                # ucode broadcasts the tile's partition 0 on hardware
                rec0 = work.tile([1, HL * 512], F32, name="rec0", tag="ysb2", bufs=1)
                nc.sync.dma_start(rec0[:], rec[64:65, :])
                rb = work.tile([64, HL * 512], F32, name="recb", tag="recb", bufs=1)
                nc.gpsimd.partition_broadcast(rb[:], rec0[:])
                cn = work.tile([64, HL * 512], BF16, name="cn", tag="cn", bufs=2)
                nc.vector.tensor_mul(cn[:], cq[0:64, :], rb[:])
                if debug and qb == 0:
                    nc.sync.dma_start(dbg["rec"][:], rec[64:65, :])
                    nc.sync.dma_start(dbg["recb"][:], rb[:])
                    nc.gpsimd.dma_start(dbg["cn"][:], cn[:])

                for g in range(2):  # two groups of 4 output tiles
                    op = psum.tile([128, 2048], F32, name="psb", tag="psb", bufs=1)
                    for ot_l in range(4):
                        ot = g * 4 + ot_l
                        for h in range(HL):
                            nc.tensor.matmul(
                                op[:, ot_l * 512:(ot_l + 1) * 512],
                                wo[h][:, ot * 128:(ot + 1) * 128],
                                cn[:, h * 512:(h + 1) * 512],
                                start=(h == 0), stop=(h == HL - 1),
                            )
                    ysb = work.tile([128, 2048], F32, name="ysb", tag="ysb", bufs=2)
                    nc.vector.tensor_copy(ysb[:], op[:])
                    nc.sync.dma_start(
                        yT_d[g * 512:(g + 1) * 512,
                             qb * 512:(qb + 1) * 512].rearrange("(o r) c -> r o c", o=4),
                        ysb.rearrange("r (o c) -> r o c", o=4),
                    )
    nc.compile()
    return nc


def _get_nc(debug=False):
    key = ("nc", debug)
    if key not in _CACHE:
        _CACHE[key] = _build_nc(debug)
    return _CACHE[key]


def kernel(x, mask, w_qkv, b_qkv, w_o, b_o):
    x = np.asarray(x, dtype=np.float32)
    mask = np.asarray(mask)
    w_qkv = np.asarray(w_qkv, dtype=np.float32)
    b_qkv = np.asarray(b_qkv, dtype=np.float32)
    w_o = np.asarray(w_o, dtype=np.float32)
    b_o = np.asarray(b_o, dtype=np.float32)
    assert not b_qkv.any(), "kernel specialized for zero qkv bias"

    scale = np.float32(1.0 / np.sqrt(HD))
    maskT = np.ascontiguousarray(mask.reshape(S, S).T).astype(ml_dtypes.bfloat16)

    w3 = w_qkv.reshape(H, 3, HD, D)  # [head, (q,k,v), hd, D]
    in_maps = []
    for c in range(N_CORES):
        b = c // 4
        h0 = (c % 4) * HL
        heads = list(range(h0, h0 + HL))
        wq = w3[heads, 0].reshape(CH, D) * scale
        wk = w3[heads, 1].reshape(CH, D)
        wv = w3[heads, 2].reshape(CH, D)
        wo_cols = np.concatenate([w_o[:, h * HD:(h + 1) * HD] for h in heads], axis=1)
        in_maps.append({
            "xT": np.ascontiguousarray(x[b].T),
            "maskT": maskT,
            "wqT": np.ascontiguousarray(wq.T),
            "wkT": np.ascontiguousarray(wk.T),
            "wvT": np.ascontiguousarray(wv.T),
            "woT": np.ascontiguousarray(wo_cols.T).astype(ml_dtypes.bfloat16),
        })

    nc = _get_nc()
    trace = bool(int(os.environ.get("MHA_TRACE", "0")))
    res = run_bass_kernel_spmd(nc, in_maps, core_ids=list(range(N_CORES)),
                               trace=trace)
    _CACHE["last_results"] = res

    y = np.zeros((B, S, D), dtype=np.float32)
    for c in range(N_CORES):
        y[c // 4] += res.results[c]["yT"].T
    y += b_o
    return y



# revision 4
# speedup vs baseline: 1.1040x; 1.1040x over previous
"""Multi-head attention (B=2, S=2048, D=1024, H=16) on 8 TRN2 NeuronCores.

Sharding: (batch, head-group) SPMD. Core c handles batch b = c//4 and local
heads [4*(c%4), 4*(c%4)+4). Each core computes its 4 heads' attention plus the
partial o-projection (row-parallel over the head dimension); the host sums the
4 partial outputs per batch and adds b_o.

All DRAM inputs are bf16 (halves load DMA); PSUM accumulation is fp32.

Phase 2 is software-pipelined over a flat iteration space i = (qb, kt, pair):
  S(i)  PE : scores S.T block  = KT_h.T @ QT_h  -> tq PSUM [128,1024]
  E(i)  ACT: ex = exp(tq)                       -> SBUF bf16
  M(i)  DVE: pt = ex * maskT                    -> SBUF bf16
  P(i)  PE : cq += [V_h|1].T @ pt               (PSUM accumulate, row 64=den)
P lags S by 4+5*qb groups so the PE never waits on the exp/mask chain; the
5-group P-gap at each qb boundary absorbs the softmax-denominator chain
(DVE reciprocal -> DMA hop -> gpsimd partition broadcast -> cn mul) and the
previous qb's o_proj matmuls, which share the tq PSUM tag.
"""
import os
import sys

if "/opt/trn_rl_repo" not in sys.path:
    sys.path.insert(0, "/opt/trn_rl_repo")
os.environ.setdefault("JAX_PLATFORMS", "axon,cpu")

from collections import defaultdict
from contextlib import ExitStack

import ml_dtypes
import numpy as np

import concourse.bass as bass
import concourse.tile as tile
from concourse import bacc, library_config, mybir
from concourse.bass_utils import run_bass_kernel_spmd

F32 = mybir.dt.float32
BF16 = mybir.dt.bfloat16
EXP = mybir.ActivationFunctionType.Exp
LN = mybir.ActivationFunctionType.Ln

B, S, D = 2, 2048, 1024
H, HD = 16, 64
HL = 4            # local heads per core
CH = HL * HD      # 256 local channels
N_CORES = 8
KC = D // 128     # 8 contraction chunks for the projections
NQB = S // 512    # 4 q blocks
NKT = S // 128    # 16 k tiles
NIT = NQB * NKT * 2   # 128 pipeline iterations (qb, kt, pair)
PT_BUFS = 20
PGAP = 5          # extra P-lag added per qb boundary

_CACHE = {}


def _build_nc():
    nc = bacc.Bacc("TRN2", target_bir_lowering=False)
    xT_d = nc.declare_dram_parameter("xT", [D, S], BF16, isOutput=False)
    mk_d = nc.declare_dram_parameter("maskT", [S, S], BF16, isOutput=False)
    wqT_d = nc.declare_dram_parameter("wqT", [D, CH], BF16, isOutput=False)
    wkT_d = nc.declare_dram_parameter("wkT", [D, CH], BF16, isOutput=False)
    wvT_d = nc.declare_dram_parameter("wvT", [D, CH], BF16, isOutput=False)
    woT_d = nc.declare_dram_parameter("woT", [CH, D], BF16, isOutput=False)
    yT_d = nc.declare_dram_parameter("yT", [D, S], F32, isOutput=True)

    with tile.TileContext(nc) as tc, ExitStack() as ctx:
        nc.gpsimd.load_library(library_config.attn)
        const = ctx.enter_context(tc.tile_pool(name="const", bufs=1))
        psum = ctx.enter_context(tc.tile_pool(name="psum", bufs=1, space="PSUM"))

        # ---- resident tensors ----
        mk = [const.tile([128, S], BF16, name=f"mk{kt}") for kt in range(NKT)]
        wo = []
        for h in range(HL):
            t = const.tile([64, D], BF16, name=f"wo{h}")
            nc.sync.dma_start(t[:], woT_d[h * 64:(h + 1) * 64, :])
            wo.append(t)
        # persistent QT/KT ([2 heads * 64 d, seq] pair tiles) and V chunks
        qt = [const.tile([128, S], BF16, name=f"qt{i}") for i in range(2)]
        kt_sb = [const.tile([128, S], BF16, name=f"kt{i}") for i in range(2)]
        v_sb = [const.tile([128, HL * 65], BF16, name=f"v{i}") for i in range(NKT)]
        for st in range(NKT):
            # ones column per head (softmax denominator trick)
            nc.gpsimd.memset(
                v_sb[st].rearrange("p (h c) -> p h c", h=HL)[:, :, 64:65], 1.0
            )

        # ---- phase 1: projections (own pool, closed before phase 2) ----
        with tc.tile_pool(name="p1", bufs=1) as p1:
            wq_sb = [p1.tile([128, CH], BF16, name=f"wq{k}") for k in range(KC)]
            wk_sb = [p1.tile([128, CH], BF16, name=f"wk{k}") for k in range(KC)]
            wv_sb = [p1.tile([128, CH], BF16, name=f"wv{k}") for k in range(KC)]
            for k in range(KC):
                nc.sync.dma_start(wq_sb[k][:], wqT_d[k * 128:(k + 1) * 128, :])
                nc.sync.dma_start(wk_sb[k][:], wkT_d[k * 128:(k + 1) * 128, :])
                nc.sync.dma_start(wv_sb[k][:], wvT_d[k * 128:(k + 1) * 128, :])

            for qh in range(4):  # seq quarters of 512
                xt = []
                for k in range(KC):
                    t = p1.tile([128, 512], BF16, name=f"xt{k}", bufs=1)
                    nc.sync.dma_start(
                        t[:], xT_d[k * 128:(k + 1) * 128, qh * 512:(qh + 1) * 512]
                    )
                    xt.append(t)

                # interleave Q/K m-tiles with V seq-tiles for PE overlap
                for j, (wsb, dst, mt) in enumerate(
                    [(wq_sb, qt, 0), (wq_sb, qt, 1), (wk_sb, kt_sb, 0), (wk_sb, kt_sb, 1)]
                ):
                    ps = psum.tile([128, 512], F32, name="psa", tag="psa", bufs=2)
                    for k in range(KC):
                        nc.tensor.matmul(
                            ps[:],
                            wsb[k][:, mt * 128:(mt + 1) * 128],
                            xt[k][:],
                            start=(k == 0), stop=(k == KC - 1),
                        )
                    nc.scalar.copy(dst[mt][:, qh * 512:(qh + 1) * 512], ps[:])
                    if j % 2 == 0:  # 2 V seq-tiles after every other QK job
                        for st_l in range(2):
                            sl = j + st_l
                            st = qh * 4 + sl
                            vp = psum.tile([128, CH], F32, name="psb", tag="psb", bufs=1)
                            for k in range(KC):
                                nc.tensor.matmul(
                                    vp[:],
                                    xt[k][:, sl * 128:(sl + 1) * 128],
                                    wv_sb[k][:],
                                    start=(k == 0), stop=(k == KC - 1),
                                )
                            nc.vector.tensor_copy(
                                v_sb[st].rearrange("p (h c) -> p h c", h=HL)[:, :, 0:64],
                                vp.rearrange("p (h c) -> p h c", h=HL),
                            )

        # mask loads issued after phase-1 inputs: first consumer is phase 2
        for kt in range(NKT):
            nc.sync.dma_start(mk[kt][:], mk_d[kt * 128:(kt + 1) * 128, :])

        # ---- phase 2: software-pipelined attention + o_proj ----
        def it_decode(i):
            return i // 32, (i // 2) % 16, i % 2   # qb, ktile, pair

        sched = defaultdict(list)
        for i in range(NIT):
            qb = i // 32
            sched[i].append(("S", i))
            sched[i + 1].append(("E", i))
            sched[i + 2].append(("M", i))
            sched[i + 4 + PGAP * qb].append(("P", i))
        for qb in range(NQB):
            lp = (qb * 32 + 31) + 4 + PGAP * qb   # group of last P of this qb
            sched[lp + 1].append(("R", qb))
            sched[lp + 5].append(("CN", qb))
            for g4 in range(4):
                sched[lp + 5 + 2 * g4].append(("O", qb, g4))
        ngroups = max(sched) + 1

        with tc.tile_pool(name="work", bufs=1) as work:
            tq_t, ex_t, pt_t, cq_t, cn_t = {}, {}, {}, {}, {}
            for g in range(ngroups):
                for op in sched[g]:
                    kind = op[0]
                    if kind == "S":
                        i = op[1]
                        qb, ktile, pair = it_decode(i)
                        tq = psum.tile([128, 1024], F32, name="psa", tag="psa", bufs=2)
                        for hh in range(2):
                            nc.tensor.matmul(
                                tq[:, hh * 512:(hh + 1) * 512],
                                kt_sb[pair][hh * 64:(hh + 1) * 64,
                                            ktile * 128:(ktile + 1) * 128],
                                qt[pair][hh * 64:(hh + 1) * 64,
                                         qb * 512:(qb + 1) * 512],
                                start=True, stop=True,
                            )
                        tq_t[i] = tq
                    elif kind == "E":
                        i = op[1]
                        ex = work.tile([128, 1024], BF16, name="expq", tag="expq", bufs=4)
                        nc.scalar.activation(ex[:], tq_t.pop(i)[:], EXP)
                        ex_t[i] = ex
                    elif kind == "M":
                        i = op[1]
                        qb, ktile, pair = it_decode(i)
                        ex = ex_t.pop(i)
                        pt = work.tile([128, 1024], BF16, name="pt", tag="pt",
                                       bufs=PT_BUFS)
                        for hh in range(2):
                            nc.vector.tensor_mul(
                                pt[:, hh * 512:(hh + 1) * 512],
                                ex[:, hh * 512:(hh + 1) * 512],
                                mk[ktile][:, qb * 512:(qb + 1) * 512],
                            )
                        pt_t[i] = pt
                    elif kind == "P":
                        i = op[1]
                        qb, ktile, pair = it_decode(i)
                        if i % 32 == 0:
                            cq_t[qb] = psum.tile([128, 2048], F32, name="psb",
                                                 tag="psb", bufs=1)
                        cq = cq_t[qb]
                        pt = pt_t.pop(i)
                        for hh in range(2):
                            h = pair * 2 + hh
                            nc.tensor.matmul(
                                cq[0:65, h * 512:(h + 1) * 512],
                                v_sb[ktile][:, h * 65:h * 65 + 65],
                                pt[:, hh * 512:(hh + 1) * 512],
                                start=(ktile == 0), stop=(ktile == NKT - 1),
                            )
                    elif kind == "R":
                        qb = op[1]
                        cq = cq_t[qb]
                        # rec = exp(-ln(den)) = 1/den; hop row to partition 0
                        # via DMA; gpsimd broadcasts partition 0 to all 64
                        nc.scalar.activation(cq[64:65, :], cq[64:65, :], LN)
                        rec = work.tile([65, 2048], F32, name="rec", tag="rec", bufs=1)
                        nc.scalar.activation(rec[64:65, :], cq[64:65, :], EXP,
                                             scale=-1.0)
                        rec0 = work.tile([1, 2048], F32, name="rec0", tag="rec0", bufs=1)
                        nc.sync.dma_start(rec0[:], rec[64:65, :])
                        rb = work.tile([64, 2048], F32, name="recb", tag="recb", bufs=1)
                        nc.gpsimd.partition_broadcast(rb[:], rec0[:])
                        _CACHE.setdefault("rb_t", {})[qb] = (rec, rec0, rb)
                    elif kind == "CN":
                        qb = op[1]
                        rec, rec0, rb = _CACHE["rb_t"].pop(qb)
                        cq = cq_t.pop(qb)
                        cn = work.tile([64, 2048], BF16, name="cn", tag="cn", bufs=1)
                        nc.vector.tensor_mul(cn[:], cq[0:64, :], rb[:])
                        cn_t[qb] = cn
                    elif kind == "O":
                        qb, g4 = op[1], op[2]
                        cn = cn_t[qb]
                        opp = psum.tile([128, 1024], F32, name="psa", tag="psa", bufs=2)
                        for ot_l in range(2):
                            ot = 2 * g4 + ot_l
                            for h in range(HL):
                                nc.tensor.matmul(
                                    opp[:, ot_l * 512:(ot_l + 1) * 512],
                                    wo[h][:, ot * 128:(ot + 1) * 128],
                                    cn[:, h * 512:(h + 1) * 512],
                                    start=(h == 0), stop=(h == HL - 1),
                                )
                        ysb = work.tile([128, 1024], F32, name="ysb", tag="ysb", bufs=2)
                        nc.vector.tensor_copy(ysb[:], opp[:])
                        nc.sync.dma_start(
                            yT_d[g4 * 256:(g4 + 1) * 256,
                                 qb * 512:(qb + 1) * 512].rearrange(
                                     "(o r) c -> r o c", o=2),
                            ysb.rearrange("r (o c) -> r o c", o=2),
                        )
                        if g4 == 3:
                            cn_t.pop(qb)
    nc.compile()
    return nc


def _get_nc():
    if "nc" not in _CACHE:
        _CACHE["nc"] = _build_nc()
    return _CACHE["nc"]


def kernel(x, mask, w_qkv, b_qkv, w_o, b_o):
    x = np.asarray(x, dtype=np.float32)
    mask = np.asarray(mask)
    w_qkv = np.asarray(w_qkv, dtype=np.float32)
    b_qkv = np.asarray(b_qkv, dtype=np.float32)
    w_o = np.asarray(w_o, dtype=np.float32)
    b_o = np.asarray(b_o, dtype=np.float32)
    assert not b_qkv.any(), "kernel specialized for zero qkv bias"

    scale = np.float32(1.0 / np.sqrt(HD))
    maskT = np.ascontiguousarray(mask.reshape(S, S).T).astype(ml_dtypes.bfloat16)

    w3 = w_qkv.reshape(H, 3, HD, D)  # [head, (q,k,v), hd, D]
    in_maps = []
    for c in range(N_CORES):
        b = c // 4
        h0 = (c % 4) * HL
        heads = list(range(h0, h0 + HL))
        wq = w3[heads, 0].reshape(CH, D) * scale
        wk = w3[heads, 1].reshape(CH, D)
        wv = w3[heads, 2].reshape(CH, D)
        wo_cols = np.concatenate([w_o[:, h * HD:(h + 1) * HD] for h in heads], axis=1)
        in_maps.append({
            "xT": np.ascontiguousarray(x[b].T).astype(ml_dtypes.bfloat16),
            "maskT": maskT,
            "wqT": np.ascontiguousarray(wq.T).astype(ml_dtypes.bfloat16),
            "wkT": np.ascontiguousarray(wk.T).astype(ml_dtypes.bfloat16),
            "wvT": np.ascontiguousarray(wv.T).astype(ml_dtypes.bfloat16),
            "woT": np.ascontiguousarray(wo_cols.T).astype(ml_dtypes.bfloat16),
        })

    nc = _get_nc()
    trace = bool(int(os.environ.get("MHA_TRACE", "0")))
    res = run_bass_kernel_spmd(nc, in_maps, core_ids=list(range(N_CORES)),
                               trace=trace)
    _CACHE["last_results"] = res

    y = np.zeros((B, S, D), dtype=np.float32)
    for c in range(N_CORES):
        y[c // 4] += res.results[c]["yT"].T
    y += b_o
    return y


# revision 13
# speedup vs baseline: 1.2715x; 1.1517x over previous
"""Multi-head attention (B=2, S=2048, D=1024, H=16) on 8 TRN2 NeuronCores.

Sharding: (batch, head-group) SPMD. Core c handles batch b = c//4 and local
heads [4*(c%4), 4*(c%4)+4). Each core computes its 4 heads' attention plus the
partial o-projection (row-parallel over the head dimension); the host sums the
4 partial outputs per batch and adds b_o.

All DRAM inputs are bf16 (halves load DMA); PSUM accumulation is fp32.

Phase 2 is software-pipelined over a flat iteration space i = (qb, kt, pair):
  S(i)  PE : scores S.T block  = KT_h.T @ QT_h  -> tq PSUM [128,1024]
  E(i)  ACT: ex = exp(tq)                       -> SBUF bf16
  M(i)  DVE: pt = ex * maskT                    -> SBUF bf16
  P(i)  PE : cq += [V_h|1].T @ pt               (PSUM accumulate, row 64=den)
P lags S by 4+5*qb groups so the PE never waits on the exp/mask chain; the
5-group P-gap at each qb boundary absorbs the softmax-denominator chain
(DVE reciprocal -> DMA hop -> gpsimd partition broadcast -> cn mul) and the
previous qb's o_proj matmuls, which share the tq PSUM tag.
"""
import os
import sys

if "/opt/trn_rl_repo" not in sys.path:
    sys.path.insert(0, "/opt/trn_rl_repo")
os.environ.setdefault("JAX_PLATFORMS", "axon,cpu")

from collections import defaultdict
from contextlib import ExitStack

import ml_dtypes
import numpy as np

import concourse.bass as bass
import concourse.tile as tile
from concourse import bacc, library_config, mybir
from concourse.bass_utils import run_bass_kernel_spmd

F32 = mybir.dt.float32
BF16 = mybir.dt.bfloat16
EXP = mybir.ActivationFunctionType.Exp
LN = mybir.ActivationFunctionType.Ln

B, S, D = 2, 2048, 1024
H, HD = 16, 64
HL = 4            # local heads per core
CH = HL * HD      # 256 local channels
N_CORES = 8
KC = D // 128     # 8 contraction chunks for the projections
NQB = S // 512    # 4 q blocks
NKT = S // 128    # 16 k tiles
NIT = NQB * NKT * 2   # 128 pipeline iterations (qb, kt, pair)
PT_BUFS = 18
PGAP = 6          # extra P-lag added per qb boundary

_CACHE = {}


def _build_nc():
    nc = bacc.Bacc("TRN2", target_bir_lowering=False)
    xT_d = nc.declare_dram_parameter("xT", [D, S], BF16, isOutput=False)
    mk_d = nc.declare_dram_parameter("maskT", [S, S], BF16, isOutput=False)
    wqkvT_d = nc.declare_dram_parameter("wqkvT", [D, 3 * CH], BF16, isOutput=False)
    woT_d = nc.declare_dram_parameter("woT", [CH, D], BF16, isOutput=False)
    yT_d = nc.declare_dram_parameter("yT", [D, S], F32, isOutput=True)

    with tile.TileContext(nc) as tc, ExitStack() as ctx:
        nc.gpsimd.load_library(library_config.attn)
        const = ctx.enter_context(tc.tile_pool(name="const", bufs=1))
        psum = ctx.enter_context(tc.tile_pool(name="psum", bufs=1, space="PSUM"))

        # ---- resident tensors ----
        mk = [const.tile([128, S], BF16, name=f"mk{kt}") for kt in range(NKT)]
        # wo2[j]: o-proj weights for head pair j, 2 heads stacked in partitions
        wo2 = []
        for j in range(2):
            t = const.tile([128, D], BF16, name=f"wo{j}")
            nc.sync.dma_start(t[:], woT_d[j * 128:(j + 1) * 128, :])
            wo2.append(t)
        # persistent QT/KT ([2 heads * 64 d, seq] pair tiles) and V chunks
        qt = [const.tile([128, S], BF16, name=f"qt{i}") for i in range(2)]
        kt_sb = [const.tile([128, S], BF16, name=f"kt{i}") for i in range(2)]
        v_sb = [const.tile([128, HL * 65], BF16, name=f"v{i}") for i in range(NKT)]
        for st in range(NKT):
            # ones column per head (softmax denominator trick)
            nc.gpsimd.memset(
                v_sb[st].rearrange("p (h c) -> p h c", h=HL)[:, :, 64:65], 1.0
            )

        # ---- phase 1: projections (own pool, closed before phase 2) ----
        with tc.tile_pool(name="p1", bufs=1) as p1:
            # one wide DMA per 128-row chunk (q|k|v side by side: 1.5KB lines)
            wsb = [p1.tile([128, 3 * CH], BF16, name=f"w{k}") for k in range(KC)]
            for k in range(KC):
                nc.sync.dma_start(wsb[k][:], wqkvT_d[k * 128:(k + 1) * 128, :])
            # resident x.T: full-seq rows per chunk (4KB lines)
            xt = [p1.tile([128, S], BF16, name=f"xt{k}") for k in range(KC)]
            for k in range(KC):
                nc.sync.dma_start(xt[k][:], xT_d[k * 128:(k + 1) * 128, :])

            for qh in range(4):  # seq quarters of 512
                # interleave Q/K m-tiles with V seq-tiles for PE overlap
                for j, (wof, dst, mt) in enumerate(
                    [(0, qt, 0), (0, qt, 1), (CH, kt_sb, 0), (CH, kt_sb, 1)]
                ):
                    ps = psum.tile([128, 512], F32, name="psa", tag="psa", bufs=2)
                    for k in range(KC):
                        nc.tensor.matmul(
                            ps[:],
                            wsb[k][:, wof + mt * 128:wof + (mt + 1) * 128],
                            xt[k][:, qh * 512:(qh + 1) * 512],
                            start=(k == 0), stop=(k == KC - 1),
                        )
                    nc.scalar.copy(dst[mt][:, qh * 512:(qh + 1) * 512], ps[:])
                    if j % 2 == 0:  # 2 V seq-tiles after every other QK job
                        for st_l in range(2):
                            sl = j + st_l
                            st = qh * 4 + sl
                            vp = psum.tile([128, CH], F32, name="psb", tag="psb", bufs=1)
                            for k in range(KC):
                                nc.tensor.matmul(
                                    vp[:],
                                    xt[k][:, st * 128:(st + 1) * 128],
                                    wsb[k][:, 2 * CH:3 * CH],
                                    start=(k == 0), stop=(k == KC - 1),
                                )
                            nc.vector.tensor_copy(
                                v_sb[st].rearrange("p (h c) -> p h c", h=HL)[:, :, 0:64],
                                vp.rearrange("p (h c) -> p h c", h=HL),
                            )

        # mask loads issued after phase-1 inputs: first consumer is phase 2
        for kt in range(NKT):
            nc.sync.dma_start(mk[kt][:], mk_d[kt * 128:(kt + 1) * 128, :])

        # ---- phase 2: software-pipelined attention + o_proj ----
        def it_decode(i):
            return i // 32, (i // 2) % 16, i % 2   # qb, ktile, pair

        sched = defaultdict(list)
        for i in range(NIT):
            qb = i // 32
            sched[i].append(("S", i))
            sched[i + 1].append(("E", i))
            sched[i + 2].append(("M", i))
            sched[i + 4 + PGAP * qb].append(("P", i))
        for qb in range(NQB):
            lp = (qb * 32 + 31) + 4 + PGAP * qb   # group of last P of this qb
            sched[lp + 1].append(("R", qb))
            sched[lp + 3].append(("CN", qb))
            for g4 in range(4):
                sched[lp + 6 + 2 * g4].append(("O", qb, g4))
        ngroups = max(sched) + 1

        with tc.tile_pool(name="work", bufs=1) as work:
            tq_t, ex_t, pt_t, cq_t, cn_t = {}, {}, {}, {}, {}
            for g in range(ngroups):
                for op in sched[g]:
                    kind = op[0]
                    if kind == "S":
                        i = op[1]
                        qb, ktile, pair = it_decode(i)
                        tq = psum.tile([128, 1024], F32, name="psa", tag="psa", bufs=2)
                        for hh in range(2):
                            nc.tensor.matmul(
                                tq[:, hh * 512:(hh + 1) * 512],
                                kt_sb[pair][hh * 64:(hh + 1) * 64,
                                            ktile * 128:(ktile + 1) * 128],
                                qt[pair][hh * 64:(hh + 1) * 64,
                                         qb * 512:(qb + 1) * 512],
                                start=True, stop=True,
                            )
                        tq_t[i] = tq
                    elif kind == "E":
                        i = op[1]
                        ex = work.tile([128, 1024], BF16, name="expq", tag="expq", bufs=4)
                        nc.scalar.activation(ex[:], tq_t.pop(i)[:], EXP)
                        ex_t[i] = ex
                    elif kind == "M":
                        i = op[1]
                        qb, ktile, pair = it_decode(i)
                        ex = ex_t.pop(i)
                        pt = work.tile([128, 1024], BF16, name="pt", tag="pt",
                                       bufs=PT_BUFS)
                        for hh in range(2):
                            nc.vector.tensor_mul(
                                pt[:, hh * 512:(hh + 1) * 512],
                                ex[:, hh * 512:(hh + 1) * 512],
                                mk[ktile][:, qb * 512:(qb + 1) * 512],
                            )
                        pt_t[i] = pt
                    elif kind == "P":
                        i = op[1]
                        qb, ktile, pair = it_decode(i)
                        if i % 32 == 0:
                            cq_t[qb] = psum.tile([128, 2048], F32, name="psb",
                                                 tag="psb", bufs=1)
                        cq = cq_t[qb]
                        pt = pt_t.pop(i)
                        for hh in range(2):
                            h = pair * 2 + hh
                            nc.tensor.matmul(
                                cq[0:65, h * 512:(h + 1) * 512],
                                v_sb[ktile][:, h * 65:h * 65 + 65],
                                pt[:, hh * 512:(hh + 1) * 512],
                                start=(ktile == 0), stop=(ktile == NKT - 1),
                            )
                    elif kind == "R":
                        qb = op[1]
                        cq = cq_t[qb]
                        # den row -> SBUF (ACT Copy: no table swap), hop to
                        # partition 0, 1/den on DVE, gpsimd-broadcast to 64
                        den = work.tile([65, 2048], F32, name="den", tag="den", bufs=1)
                        nc.scalar.copy(den[64:65, :], cq[64:65, :])
                        den0 = work.tile([1, 2048], F32, name="den0", tag="den0", bufs=1)
                        nc.sync.dma_start(den0[:], den[64:65, :])
                        rec0 = work.tile([1, 2048], F32, name="rec0", tag="rec0", bufs=1)
                        nc.vector.reciprocal_approx_fast(rec0[:], den0[:])
                        rb = work.tile([64, 2048], F32, name="recb", tag="recb", bufs=1)
                        nc.gpsimd.partition_broadcast(rb[:], rec0[:])
                        _CACHE.setdefault("rb_t", {})[qb] = (den, den0, rec0, rb)
                    elif kind == "CN":
                        qb = op[1]
                        den, den0, rec0, rb = _CACHE["rb_t"].pop(qb)
                        cq = cq_t.pop(qb)
                        # cn2: even heads (0,2) in partitions 0:64, odd heads
                        # (1,3) DMA-shifted to 64:128; free = [pair j, 512 q]
                        cq4 = cq[0:64, :].rearrange("p (h c) -> p h c", h=HL)
                        rb4 = rb.rearrange("p (h c) -> p h c", h=HL)
                        cn2 = work.tile([128, 1024], BF16, name="cn2", tag="cn2", bufs=1)
                        cno = work.tile([64, 1024], BF16, name="cno", tag="cno", bufs=1)
                        nc.vector.tensor_mul(
                            cn2[0:64, :].rearrange("p (j c) -> p j c", j=2),
                            cq4[:, 0::2, :], rb4[:, 0::2, :],
                        )
                        nc.vector.tensor_mul(
                            cno.rearrange("p (j c) -> p j c", j=2),
                            cq4[:, 1::2, :], rb4[:, 1::2, :],
                        )
                        nc.sync.dma_start(cn2[64:128, :], cno[:])
                        cn_t[qb] = cn2
                    elif kind == "O":
                        qb, g4 = op[1], op[2]
                        cn2 = cn_t[qb]
                        opp = psum.tile([128, 1024], F32, name="psa", tag="psa", bufs=2)
                        for ot_l in range(2):
                            ot = 2 * g4 + ot_l
                            for j in range(2):
                                nc.tensor.matmul(
                                    opp[:, ot_l * 512:(ot_l + 1) * 512],
                                    wo2[j][:, ot * 128:(ot + 1) * 128],
                                    cn2[:, j * 512:(j + 1) * 512],
                                    start=(j == 0), stop=(j == 1),
                                )
                        ysb = work.tile([128, 1024], F32, name="ysb", tag="ysb", bufs=2)
                        nc.vector.tensor_copy(ysb[:], opp[:])
                        nc.sync.dma_start(
                            yT_d[g4 * 256:(g4 + 1) * 256,
                                 qb * 512:(qb + 1) * 512].rearrange(
                                     "(o r) c -> r o c", o=2),
                            ysb.rearrange("r (o c) -> r o c", o=2),
                        )
                        if g4 == 3:
                            cn_t.pop(qb)
    nc.compile()
    return nc


def _get_nc():
    if "nc" not in _CACHE:
        _CACHE["nc"] = _build_nc()
    return _CACHE["nc"]


def kernel(x, mask, w_qkv, b_qkv, w_o, b_o):
    x = np.asarray(x, dtype=np.float32)
    mask = np.asarray(mask)
    w_qkv = np.asarray(w_qkv, dtype=np.float32)
    b_qkv = np.asarray(b_qkv, dtype=np.float32)
    w_o = np.asarray(w_o, dtype=np.float32)
    b_o = np.asarray(b_o, dtype=np.float32)
    assert not b_qkv.any(), "kernel specialized for zero qkv bias"

    scale = np.float32(1.0 / np.sqrt(HD))
    maskT = np.ascontiguousarray(mask.reshape(S, S).T).astype(ml_dtypes.bfloat16)

    w3 = w_qkv.reshape(H, 3, HD, D)  # [head, (q,k,v), hd, D]
    in_maps = []
    for c in range(N_CORES):
        b = c // 4
        h0 = (c % 4) * HL
        heads = list(range(h0, h0 + HL))
        wq = w3[heads, 0].reshape(CH, D) * scale
        wk = w3[heads, 1].reshape(CH, D)
        wv = w3[heads, 2].reshape(CH, D)
        wqkv = np.concatenate([wq.T, wk.T, wv.T], axis=1)  # [D, 3CH]
        wo_cols = np.concatenate([w_o[:, h * HD:(h + 1) * HD] for h in heads], axis=1)
        in_maps.append({
            "xT": np.ascontiguousarray(x[b].T).astype(ml_dtypes.bfloat16),
            "maskT": maskT,
            "wqkvT": np.ascontiguousarray(wqkv).astype(ml_dtypes.bfloat16),
            "woT": np.ascontiguousarray(wo_cols.T).astype(ml_dtypes.bfloat16),
        })

    nc = _get_nc()
    trace = bool(int(os.environ.get("MHA_TRACE", "0")))
    res = run_bass_kernel_spmd(nc, in_maps, core_ids=list(range(N_CORES)),
                               trace=trace)
    _CACHE["last_results"] = res

    y = np.zeros((B, S, D), dtype=np.float32)
    for c in range(N_CORES):
        y[c // 4] += res.results[c]["yT"].T
    y += b_o
    return y


# revision 17
# speedup vs baseline: 1.3498x; 1.0616x over previous
"""Multi-head attention (B=2, S=2048, D=1024, H=16) on 8 TRN2 NeuronCores.

Sharding: (batch, head-group) SPMD. Core c handles batch b = c//4 and local
heads [4*(c%4), 4*(c%4)+4). Each core computes its 4 heads' attention plus the
partial o-projection (row-parallel over the head dimension); the host sums the
4 partial outputs per batch and adds b_o.

All DRAM inputs are bf16 (halves load DMA); PSUM accumulation is fp32.

Phase 2 is software-pipelined over a flat iteration space i = (qb, kt, pair):
  S(i)  PE : scores S.T block  = KT_h.T @ QT_h  -> tq PSUM [128,1024]
  E(i)  ACT: ex = exp(tq)                       -> SBUF bf16
  M(i)  DVE: pt = ex * maskT                    -> SBUF bf16
  P(i)  PE : cq += [V_h|1].T @ pt               (PSUM accumulate, row 64=den)
P lags S by 4+5*qb groups so the PE never waits on the exp/mask chain; the
5-group P-gap at each qb boundary absorbs the softmax-denominator chain
(DVE reciprocal -> DMA hop -> gpsimd partition broadcast -> cn mul) and the
previous qb's o_proj matmuls, which share the tq PSUM tag.
"""
import os
import sys

if "/opt/trn_rl_repo" not in sys.path:
    sys.path.insert(0, "/opt/trn_rl_repo")
os.environ.setdefault("JAX_PLATFORMS", "axon,cpu")

from collections import defaultdict
from contextlib import ExitStack

import ml_dtypes
import numpy as np

import concourse.bass as bass
import concourse.tile as tile
from concourse import bacc, library_config, mybir
from concourse.bass_utils import run_bass_kernel_spmd

F32 = mybir.dt.float32
BF16 = mybir.dt.bfloat16
EXP = mybir.ActivationFunctionType.Exp
LN = mybir.ActivationFunctionType.Ln

B, S, D = 2, 2048, 1024
H, HD = 16, 64
HL = 4            # local heads per core
CH = HL * HD      # 256 local channels
N_CORES = 8
KC = D // 128     # 8 contraction chunks for the projections
NQB = S // 512    # 4 q blocks
NKT = S // 128    # 16 k tiles
NIT = NQB * NKT * 2   # 128 pipeline iterations (qb, kt, pair)
PT_BUFS = 12
PGAP = 2          # extra P-lag added per qb boundary

_CACHE = {}


def _build_nc():
    nc = bacc.Bacc("TRN2", target_bir_lowering=False)
    xT_d = nc.declare_dram_parameter("xT", [D, S], BF16, isOutput=False)
    mk_d = nc.declare_dram_parameter("maskT", [S, S], BF16, isOutput=False)
    wqkvT_d = nc.declare_dram_parameter("wqkvT", [D, 3 * CH], BF16, isOutput=False)
    woT_d = nc.declare_dram_parameter("woT", [CH, D], BF16, isOutput=False)
    yT_d = nc.declare_dram_parameter("yT", [D, S], F32, isOutput=True)

    with tile.TileContext(nc) as tc, ExitStack() as ctx:
        nc.gpsimd.load_library(library_config.attn)
        const = ctx.enter_context(tc.tile_pool(name="const", bufs=1))
        psum = ctx.enter_context(tc.tile_pool(name="psum", bufs=1, space="PSUM"))

        # ---- resident tensors ----
        mk = [const.tile([128, S], BF16, name=f"mk{kt}") for kt in range(NKT)]
        # wo2[j]: o-proj weights for head pair j, 2 heads stacked in partitions
        wo2 = []
        for j in range(2):
            t = const.tile([128, D], BF16, name=f"wo{j}")
            nc.sync.dma_start(t[:], woT_d[j * 128:(j + 1) * 128, :])
            wo2.append(t)
        # persistent QT/KT ([2 heads * 64 d, seq] pair tiles) and V chunks
        qt = [const.tile([128, S], BF16, name=f"qt{i}") for i in range(2)]
        kt_sb = [const.tile([128, S], BF16, name=f"kt{i}") for i in range(2)]
        v_sb = [const.tile([128, HL * 65], BF16, name=f"v{i}") for i in range(NKT)]
        for st in range(NKT):
            # ones column per head (softmax denominator trick)
            nc.gpsimd.memset(
                v_sb[st].rearrange("p (h c) -> p h c", h=HL)[:, :, 64:65], 1.0
            )

        # ---- phase 1: projections (own pool, closed before phase 2) ----
        with tc.tile_pool(name="p1", bufs=1) as p1:
            # one wide DMA per 128-row chunk (q|k|v side by side: 1.5KB lines)
            wsb = [p1.tile([128, 3 * CH], BF16, name=f"w{k}") for k in range(KC)]
            for k in range(KC):
                nc.sync.dma_start(wsb[k][:], wqkvT_d[k * 128:(k + 1) * 128, :])
            for qh in range(4):  # seq quarters of 512
                xt = []
                for k in range(KC):
                    t = p1.tile([128, 512], BF16, name=f"xt{k}", bufs=2)
                    nc.sync.dma_start(
                        t[:], xT_d[k * 128:(k + 1) * 128, qh * 512:(qh + 1) * 512]
                    )
                    xt.append(t)

                # interleave Q/K m-tiles with V seq-tiles for PE overlap
                for j, (wof, dst, mt) in enumerate(
                    [(0, qt, 0), (0, qt, 1), (CH, kt_sb, 0), (CH, kt_sb, 1)]
                ):
                    ps = psum.tile([128, 512], F32, name="psa", tag="psa", bufs=2)
                    for k in range(KC):
                        nc.tensor.matmul(
                            ps[:],
                            wsb[k][:, wof + mt * 128:wof + (mt + 1) * 128],
                            xt[k][:],
                            start=(k == 0), stop=(k == KC - 1),
                        )
                    nc.scalar.copy(dst[mt][:, qh * 512:(qh + 1) * 512], ps[:])
                    if j % 2 == 0:  # 2 V seq-tiles after every other QK job
                        for st_l in range(2):
                            sl = j + st_l
                            st = qh * 4 + sl
                            vp = psum.tile([128, CH], F32, name="psb", tag="psb", bufs=1)
                            for k in range(KC):
                                nc.tensor.matmul(
                                    vp[:],
                                    xt[k][:, sl * 128:(sl + 1) * 128],
                                    wsb[k][:, 2 * CH:3 * CH],
                                    start=(k == 0), stop=(k == KC - 1),
                                )
                            nc.vector.tensor_copy(
                                v_sb[st].rearrange("p (h c) -> p h c", h=HL)[:, :, 0:64],
                                vp.rearrange("p (h c) -> p h c", h=HL),
                            )

        # mask loads issued after phase-1 inputs: first consumer is phase 2
        for kt in range(NKT):
            nc.sync.dma_start(mk[kt][:], mk_d[kt * 128:(kt + 1) * 128, :])

        # ---- phase 2: software-pipelined attention + o_proj ----
        def it_decode(i):
            return i // 32, (i // 2) % 16, i % 2   # qb, ktile, pair

        sched = defaultdict(list)
        for i in range(NIT):
            qb = i // 32
            sched[i].append(("S", i))
            sched[i + 1].append(("E", i))
            sched[i + 2].append(("M", i))
            sched[i + 4 + PGAP * qb].append(("P", i))
        for qb in range(NQB):
            lp = (qb * 32 + 31) + 4 + PGAP * qb   # group of last P of this qb
            # CP frees the cq PSUM tile ~2us after the last P, so the next
            # qb's P-stream flows with only a PGAP-group bubble; the rest of
            # the chain runs off SBUF, off the critical path.
            sched[lp + 1].append(("CP", qb))
            sched[lp + 1].append(("R", qb))
            if qb < NQB - 1:
                sched[lp + 7].append(("CN", qb))
                for g4 in range(4):
                    sched[lp + 11 + 2 * g4].append(("O", qb, g4))
            else:
                sched[lp + 4].append(("CN", qb))
                for g4 in range(4):
                    sched[lp + 5 + g4].append(("O", qb, g4))
        ngroups = max(sched) + 1

        with tc.tile_pool(name="work", bufs=1) as work:
            tq_t, ex_t, pt_t, cq_t, cn_t = {}, {}, {}, {}, {}
            for g in range(ngroups):
                for op in sched[g]:
                    kind = op[0]
                    if kind == "S":
                        i = op[1]
                        qb, ktile, pair = it_decode(i)
                        tq = psum.tile([128, 1024], F32, name="psa", tag="psa", bufs=2)
                        for hh in range(2):
                            nc.tensor.matmul(
                                tq[:, hh * 512:(hh + 1) * 512],
                                kt_sb[pair][hh * 64:(hh + 1) * 64,
                                            ktile * 128:(ktile + 1) * 128],
                                qt[pair][hh * 64:(hh + 1) * 64,
                                         qb * 512:(qb + 1) * 512],
                                start=True, stop=True,
                            )
                        tq_t[i] = tq
                    elif kind == "E":
                        i = op[1]
                        ex = work.tile([128, 1024], BF16, name="expq", tag="expq", bufs=4)
                        nc.scalar.activation(ex[:], tq_t.pop(i)[:], EXP)
                        ex_t[i] = ex
                    elif kind == "M":
                        i = op[1]
                        qb, ktile, pair = it_decode(i)
                        ex = ex_t.pop(i)
                        pt = work.tile([128, 1024], BF16, name="pt", tag="pt",
                                       bufs=PT_BUFS)
                        for hh in range(2):
                            nc.vector.tensor_mul(
                                pt[:, hh * 512:(hh + 1) * 512],
                                ex[:, hh * 512:(hh + 1) * 512],
                                mk[ktile][:, qb * 512:(qb + 1) * 512],
                            )
                        pt_t[i] = pt
                    elif kind == "P":
                        i = op[1]
                        qb, ktile, pair = it_decode(i)
                        if i % 32 == 0:
                            cq_t[qb] = psum.tile([128, 2048], F32, name="psb",
                                                 tag="psb", bufs=1)
                        cq = cq_t[qb]
                        pt = pt_t.pop(i)
                        for hh in range(2):
                            h = pair * 2 + hh
                            nc.tensor.matmul(
                                cq[0:65, h * 512:(h + 1) * 512],
                                v_sb[ktile][:, h * 65:h * 65 + 65],
                                pt[:, hh * 512:(hh + 1) * 512],
                                start=(ktile == 0), stop=(ktile == NKT - 1),
                            )
                    elif kind == "CP":
                        qb = op[1]
                        cq = cq_t.pop(qb)
                        # bulk ctx+den evacuation to SBUF: frees the cq PSUM
                        # tile for the next qb's PV accumulation
                        cqs = work.tile([65, 2048], F32, name="cqs", tag="cqs", bufs=1)
                        nc.scalar.copy(cqs[:], cq[0:65, :])
                        _CACHE.setdefault("cqs_t", {})[qb] = cqs
                    elif kind == "R":
                        qb = op[1]
                        cqs = _CACHE["cqs_t"][qb]
                        # hop den row to partition 0, 1/den on DVE,
                        # gpsimd-broadcast to 64 partitions
                        den0 = work.tile([1, 2048], F32, name="den0", tag="den0", bufs=1)
                        nc.sync.dma_start(den0[:], cqs[64:65, :])
                        rec0 = work.tile([1, 2048], F32, name="rec0", tag="rec0", bufs=1)
                        nc.vector.reciprocal_approx_fast(rec0[:], den0[:])
                        rb = work.tile([64, 2048], F32, name="recb", tag="recb", bufs=1)
                        nc.gpsimd.partition_broadcast(rb[:], rec0[:])
                        _CACHE.setdefault("rb_t", {})[qb] = (den0, rec0, rb)
                    elif kind == "CN":
                        qb = op[1]
                        den0, rec0, rb = _CACHE["rb_t"].pop(qb)
                        cqs = _CACHE["cqs_t"].pop(qb)
                        # cn2: even heads (0,2) in partitions 0:64, odd heads
                        # (1,3) DMA-shifted to 64:128; free = [pair j, 512 q]
                        cq4 = cqs[0:64, :].rearrange("p (h c) -> p h c", h=HL)
                        rb4 = rb.rearrange("p (h c) -> p h c", h=HL)
                        cn2 = work.tile([128, 1024], BF16, name="cn2", tag="cn2", bufs=1)
                        cno = work.tile([64, 1024], BF16, name="cno", tag="cno", bufs=1)
                        nc.vector.tensor_mul(
                            cn2[0:64, :].rearrange("p (j c) -> p j c", j=2),
                            cq4[:, 0::2, :], rb4[:, 0::2, :],
                        )
                        nc.vector.tensor_mul(
                            cno.rearrange("p (j c) -> p j c", j=2),
                            cq4[:, 1::2, :], rb4[:, 1::2, :],
                        )
                        nc.sync.dma_start(cn2[64:128, :], cno[:])
                        cn_t[qb] = cn2
                    elif kind == "O":
                        qb, g4 = op[1], op[2]
                        cn2 = cn_t[qb]
                        opp = psum.tile([128, 1024], F32, name="psa", tag="psa", bufs=2)
                        for ot_l in range(2):
                            ot = 2 * g4 + ot_l
                            for j in range(2):
                                nc.tensor.matmul(
                                    opp[:, ot_l * 512:(ot_l + 1) * 512],
                                    wo2[j][:, ot * 128:(ot + 1) * 128],
                                    cn2[:, j * 512:(j + 1) * 512],
                                    start=(j == 0), stop=(j == 1),
                                )
                        ysb = work.tile([128, 1024], F32, name="ysb", tag="ysb", bufs=2)
                        nc.vector.tensor_copy(ysb[:], opp[:])
                        nc.sync.dma_start(
                            yT_d[g4 * 256:(g4 + 1) * 256,
                                 qb * 512:(qb + 1) * 512].rearrange(
                                     "(o r) c -> r o c", o=2),
                            ysb.rearrange("r (o c) -> r o c", o=2),
                        )
                        if g4 == 3:
                            cn_t.pop(qb)
    nc.compile()
    return nc


def _get_nc():
    if "nc" not in _CACHE:
        _CACHE["nc"] = _build_nc()
    return _CACHE["nc"]


def kernel(x, mask, w_qkv, b_qkv, w_o, b_o):
    x = np.asarray(x, dtype=np.float32)
    mask = np.asarray(mask)
    w_qkv = np.asarray(w_qkv, dtype=np.float32)
    b_qkv = np.asarray(b_qkv, dtype=np.float32)
    w_o = np.asarray(w_o, dtype=np.float32)
    b_o = np.asarray(b_o, dtype=np.float32)
    assert not b_qkv.any(), "kernel specialized for zero qkv bias"

    scale = np.float32(1.0 / np.sqrt(HD))
    maskT = np.ascontiguousarray(mask.reshape(S, S).T).astype(ml_dtypes.bfloat16)

    w3 = w_qkv.reshape(H, 3, HD, D)  # [head, (q,k,v), hd, D]
    in_maps = []
    for c in range(N_CORES):
        b = c // 4
        h0 = (c % 4) * HL
        heads = list(range(h0, h0 + HL))
        wq = w3[heads, 0].reshape(CH, D) * scale
        wk = w3[heads, 1].reshape(CH, D)
        wv = w3[heads, 2].reshape(CH, D)
        wqkv = np.concatenate([wq.T, wk.T, wv.T], axis=1)  # [D, 3CH]
        wo_cols = np.concatenate([w_o[:, h * HD:(h + 1) * HD] for h in heads], axis=1)
        in_maps.append({
            "xT": np.ascontiguousarray(x[b].T).astype(ml_dtypes.bfloat16),
            "maskT": maskT,
            "wqkvT": np.ascontiguousarray(wqkv).astype(ml_dtypes.bfloat16),
            "woT": np.ascontiguousarray(wo_cols.T).astype(ml_dtypes.bfloat16),
        })

    nc = _get_nc()
    trace = bool(int(os.environ.get("MHA_TRACE", "0")))
    res = run_bass_kernel_spmd(nc, in_maps, core_ids=list(range(N_CORES)),
                               trace=trace)
    _CACHE["last_results"] = res

    y = np.zeros((B, S, D), dtype=np.float32)
    for c in range(N_CORES):
        y[c // 4] += res.results[c]["yT"].T
    y += b_o
    return y
